# revision 50
# baseline (speedup 1.0000x reference)
"""Trainium2 Bass kernel for nn_Attention_22179211117150 (sparse axial attention).

Strategy (8 NeuronCores, zero collectives); HW ~142us vs 173us baseline:
  - Axial attention: tokens attend within their own frame (N=1024 tokens,
    F=16 frames). 2 frames per core; weights replicated; fully local.
  - Keys/values compressed on host to the kept (mask!=0) positions.
  - nk=516 kept keys split as 4 FULL 128-key tiles + nlf=4 leftover keys.
    Leftover k/v projections are computed on host (tiny); leftover sim for
    all (hp, hr) packs into ONE [128, 1024] psum tile via 64x32 PE array
    tiling (rows 32*hp+j, cols hr*512+q), so ONE exp covers what used to
    take 16 activations. Leftover psum is memset to -1e9 first so unused
    rows exp to 0, letting the leftover av ride a standard K=128 matmul
    against zero-padded vlf (no mid-group array-mode switch).
  - All matmuls bf16 (fp32 psum); softmax exp in f32 on ScalarE.
  - Transposed dataflow: qT/kT [d, tokens], simT [keys, queries].
  - Sim pairs: one [128, 1024] psum tile per (jt, iw) holds BOTH heads
    side by side (cols hr*512+q); the hr0/hr1 matmuls share the tile so
    they become ready together and issue back-to-back into different
    64-row PE tiles + banks -> concurrent (Dstart ~4ns, ~2x sim).
  - Softmax denominators: the av lhsT carries the v dims in cols 0:64 and
    ALL-ONES in cols 64:128, so den lands REPLICATED at psum rows 64:128
    for free (matmuls are N-bound; M=128 costs the same as M=65). A
    partition-shift tensor_copy (rows 64:128 -> lanes 0:63, verified
    exact) + reciprocal_approx_fast + one normalize mul finish it — no
    broadcast matmul, no DMA bounce.
    (Pitfalls found: reciprocal_approx_fast misreads PSUM rows by one
    element and breaks at nonzero SBUF base partitions — keep its input
    at SBUF partition 0.)
  - Diagonal mask: narrow band multiply on GpSimd over a [128, 2, bw]
    strided view of ET covering both heads in one instruction; band
    coords are local to the 512-wide iw window of each key tile.
  - Demand-driven emission: projection/out-proj/av work is queued as fill
    units and drained between sim key-tiles so the in-order PE queue always
    has work while ScalarE chews exps. fp8 DoubleRow q/k projections were
    measured SLOWER (160us, DR LDWEIGHTS penalty + mode switches) and are
    off; steady-state PE runs at the 216ns/MM N=512 roofline.
"""
import numpy as np
import ml_dtypes
from collections import deque
from contextlib import ExitStack

import concourse.bass as bass
import concourse.mybir as mybir
import concourse.tile as tile
from concourse import bacc
from concourse.bass_utils import run_bass_kernel_spmd

dt = mybir.dt
AF = mybir.ActivationFunctionType
bf16 = ml_dtypes.bfloat16

B, F, N, H, D, DIM = 1, 16, 1024, 8, 64, 512
NCORES = 8
FPC = F // NCORES          # frames per core
T = FPC * N                # tokens per core

TRACE = False
TRACE_TMPDIR = None
LAST = {}

import os as _os
DEN_DMA = _os.environ.get("KDEN_DMA", "1") == "1"
LF_ON = _os.environ.get("KLF_ON", "1") == "1"
FP8_QK = _os.environ.get("KFP8", "0") == "1"   # fp8 DoubleRow q/k: passes
# (1.62e-2) but measured SLOWER (160us vs 145us): DR LDWEIGHTS penalty +
# mode switches outweigh the halved MM count. Keep off.
FP8_S = 64.0               # weight pre-scale so fp8 values stay normal

_nc_cache = {}


def _build(njt, nlf, diag, band_lo, band_w, lf_bands, lf_w):
    """njt full 128-key tiles + nlf (<=32) leftover keys per frame.

    lf_bands: tuple of iw windows (0/1) that contain leftover diag
    positions; lf_w: band width of the leftover diag mask."""
    nkp = njt * 128
    KV = FPC * nkp                     # kv rows per core (no padding)
    nc = bacc.Bacc("TRN2", target_bir_lowering=False, debug=False,
                   num_devices=NCORES)

    qk_dt = dt.float8e4 if FP8_QK else dt.bfloat16
    xT_d = nc.declare_dram_parameter("xT", [128, 4 * T], qk_dt, isOutput=False)
    wq_d = nc.declare_dram_parameter("wq", [128, 4 * 512], qk_dt, isOutput=False)
    wk_d = nc.declare_dram_parameter("wk", [128, 4 * 512], qk_dt, isOutput=False)
    if FP8_QK:
        xkv8_d = nc.declare_dram_parameter("xkv8", [128, 4 * KV], dt.float8e4,
                                           isOutput=False)
    xkvT_d = nc.declare_dram_parameter("xkvT", [128, 4 * KV], dt.bfloat16, isOutput=False)
    wv_d = nc.declare_dram_parameter("wv", [128, 4 * 512], dt.bfloat16, isOutput=False)
    wo_d = nc.declare_dram_parameter("wo", [128, 4 * 512], dt.bfloat16, isOutput=False)
    if diag:
        mmb_d = nc.declare_dram_parameter("mmb", [128, njt * 2 * band_w],
                                          dt.bfloat16, isOutput=False)
    if nlf:
        klf_d = nc.declare_dram_parameter("klf", [128, 8 * nlf], dt.bfloat16,
                                          isOutput=False)
        # vlf: one [128, 128] column block per (hp, hr, f) — cols 0:64 the
        # v dims, cols 64:128 all-ones (den replication); only rows
        # 32*hp .. 32*hp+nlf are nonzero, so a standard K=128 matmul picks
        # out just that hp's leftover keys (zero rows annihilate the rest).
        vlf_d = nc.declare_dram_parameter("vlf", [128, 16 * 128], dt.bfloat16,
                                          isOutput=False)
        if diag and lf_bands:
            mlf_d = nc.declare_dram_parameter(
                "mlf", [128, len(lf_bands) * 2 * lf_w], dt.bfloat16,
                isOutput=False)
    out_d = nc.declare_dram_parameter("out", [T, DIM], dt.float32, isOutput=True)

    with tile.TileContext(nc) as tc, ExitStack() as ctx:
        consts = ctx.enter_context(tc.tile_pool(name="consts", bufs=1))
        work = ctx.enter_context(tc.tile_pool(name="work", bufs=1))
        etp = ctx.enter_context(tc.tile_pool(name="etp", bufs=12))
        lfet = ctx.enter_context(tc.tile_pool(name="lfet", bufs=4))
        smallp = ctx.enter_context(tc.tile_pool(name="small", bufs=6))
        outp = ctx.enter_context(tc.tile_pool(name="outp", bufs=3))
        dramp = ctx.enter_context(tc.tile_pool(name="dramp", bufs=4, space="DRAM"))
        simp = ctx.enter_context(tc.tile_pool(name="simp", bufs=2, space="PSUM"))
        avp = ctx.enter_context(tc.tile_pool(name="avp", bufs=2, space="PSUM"))
        pp = ctx.enter_context(tc.tile_pool(name="pp", bufs=2, space="PSUM"))

        def load(d, shape, dtype, tag, eng=None, frame_split=False):
            eng = eng or nc.sync
            t = consts.tile(shape, dtype, tag=tag, name=tag)
            n = shape[1]
            if frame_split:
                # 4 contraction chunks x FPC frame-halves; frame-0 halves
                # first so frame-0 projections start as early as possible.
                half = n // 8
                for fh in range(2):
                    for cc in range(4):
                        o = cc * (n // 4) + fh * half
                        eng.dma_start(t[:, o:o + half], d[:, o:o + half])
            else:
                eng.dma_start(t[:], d[:])
            return t

        # queue programs ordered so the FIRST group's inputs stream first:
        #   sync:   wq, xT-f0, xT-f1, wo
        #   scalar: wk, xkvT-f0, wv, xkvT-f1
        #   gpsimd: mmb, klf, vlf
        wq = load(wq_d, [128, 4 * 512], qk_dt, "wq")
        wk = load(wk_d, [128, 4 * 512], qk_dt, "wk", eng=nc.scalar)
        xT = load(xT_d, [128, 4 * T], qk_dt, "xT", frame_split=True)
        if FP8_QK:
            xkv8 = load(xkv8_d, [128, 4 * KV], dt.float8e4, "xkv8",
                        eng=nc.gpsimd, frame_split=True)
        xkvT = consts.tile([128, 4 * KV], dt.bfloat16, tag="xkvT", name="xkvT")
        kvh = KV // 2
        for cc in range(4):
            nc.scalar.dma_start(xkvT[:, cc * KV: cc * KV + kvh],
                                xkvT_d[:, cc * KV: cc * KV + kvh])
        wv = load(wv_d, [128, 4 * 512], dt.bfloat16, "wv", eng=nc.scalar)
        for cc in range(4):
            nc.scalar.dma_start(xkvT[:, cc * KV + kvh: (cc + 1) * KV],
                                xkvT_d[:, cc * KV + kvh: (cc + 1) * KV])
        wo = load(wo_d, [128, 4 * 512], dt.bfloat16, "wo")
        if diag:
            mmb = load(mmb_d, [128, njt * 2 * band_w], dt.bfloat16, "mmb",
                       eng=nc.gpsimd)
        if nlf:
            klf = load(klf_d, [128, 8 * nlf], dt.bfloat16, "klf", eng=nc.gpsimd)
            vlf = load(vlf_d, [128, 16 * 128], dt.bfloat16, "vlf", eng=nc.gpsimd)
            if diag and lf_bands:
                mlf = load(mlf_d, [128, len(lf_bands) * 2 * lf_w], dt.bfloat16,
                           "mlf", eng=nc.gpsimd)

        ones_sb = work.tile([128, 64], dt.bfloat16, tag="ones", name="ones")
        nc.vector.memset(ones_sb[:], 1.0)

        # PE warm-up burst while inputs stream in (HAM clock gate).
        warm_src = work.tile([128, 512], dt.bfloat16, tag="warmsrc", name="warmsrc")
        nc.vector.memset(warm_src[:], 0.5)
        wps = pp.tile([128, 512], dt.float32, tag="pp", name="pp_t")
        for wi in range(12):
            nc.tensor.matmul(wps[0:64, :], ones_sb[:, 0:64], warm_src[:],
                             start=(wi == 0), stop=(wi == 11))
        wsb = smallp.tile([1, 64], dt.float32, tag="warm", name="warm_t")
        nc.vector.tensor_copy(wsb[:], wps[0:1, 0:64])
        wdr = dramp.tile([1, 64], dt.float32, tag="wdr", name="wdr_t")
        nc.sync.dma_start(wdr[:], wsb[:])

        qT = [work.tile([128, T], dt.bfloat16, tag=f"qT{hp}", name=f"qT{hp}")
              for hp in range(4)]
        kT = [work.tile([128, KV], dt.bfloat16, tag=f"kT{hp}", name=f"kT{hp}")
              for hp in range(4)]
        # per head h: cols 128h..128h+64 = v dims, cols 128h+64..128h+128 =
        # ones. The av matmul then yields av at psum rows 0:64 AND the
        # softmax denominator REPLICATED at rows 64:128 — no broadcast step.
        vt = [[work.tile([128, 8 * 128], dt.bfloat16, tag=f"v{f}_{jt}",
                         name=f"v{f}_{jt}") for jt in range(njt)]
              for f in range(FPC)]
        aoT = [work.tile([128, T], dt.bfloat16, tag=f"aoT{hp}", name=f"aoT{hp}")
               for hp in range(4)]

        lf_ET = {}                     # (f, iw) -> [128, 1024] bf16 tile

        # ---- emitters ----

        DR = mybir.MatmulPerfMode.DoubleRow

        def emit_q_slice(f, hp, iw):
            w0 = f * N + iw * 512
            ps = pp.tile([128, 512], dt.float32, tag="pp", name="pp_t")
            if FP8_QK:
                for g in (0, 1):
                    w3 = wq[:, g * 1024 + hp * 256: g * 1024 + hp * 256 + 256
                            ].rearrange("p (j m) -> p j m", j=2)
                    x3 = xT[:, g * 2 * T: (g + 1) * 2 * T
                            ].rearrange("p (j t) -> p j t", j=2)
                    nc.tensor.matmul(ps[:], w3[:, :, :], x3[:, :, w0:w0 + 512],
                                     start=(g == 0), stop=(g == 1),
                                     perf_mode=DR)
                nc.vector.tensor_scalar_mul(qT[hp][:, w0:w0 + 512], ps[:],
                                            1.0 / FP8_S)
            else:
                for cc in range(4):
                    nc.tensor.matmul(
                        ps[:],
                        wq[:, cc * 512 + hp * 128: cc * 512 + hp * 128 + 128],
                        xT[:, cc * T + w0: cc * T + w0 + 512],
                        start=(cc == 0), stop=(cc == 3))
                nc.vector.tensor_copy(qT[hp][:, w0:w0 + 512], ps[:])

        def emit_k_slice(f, hp):
            c0 = f * nkp
            ps = pp.tile([128, 512], dt.float32, tag="pp", name="pp_t")
            if FP8_QK:
                for g in (0, 1):
                    w3 = wk[:, g * 1024 + hp * 256: g * 1024 + hp * 256 + 256
                            ].rearrange("p (j m) -> p j m", j=2)
                    x3 = xkv8[:, g * 2 * KV: (g + 1) * 2 * KV
                              ].rearrange("p (j t) -> p j t", j=2)
                    nc.tensor.matmul(ps[:, 0:nkp], w3[:, :, :],
                                     x3[:, :, c0:c0 + nkp],
                                     start=(g == 0), stop=(g == 1),
                                     perf_mode=DR)
                nc.vector.tensor_scalar_mul(kT[hp][:, c0:c0 + nkp],
                                            ps[:, 0:nkp], 1.0 / FP8_S)
            else:
                for cc in range(4):
                    nc.tensor.matmul(
                        ps[:, 0:nkp],
                        wk[:, cc * 512 + hp * 128: cc * 512 + hp * 128 + 128],
                        xkvT[:, cc * KV + c0: cc * KV + c0 + nkp],
                        start=(cc == 0), stop=(cc == 3))
                nc.vector.tensor_copy(kT[hp][:, c0:c0 + nkp], ps[:, 0:nkp])

        def emit_v_slice(f, jt):
            col0 = f * nkp + jt * 128
            ps = pp.tile([128, 512], dt.float32, tag="pp", name="pp_t")
            for cc in range(4):
                nc.tensor.matmul(ps[:],
                                 xkvT[:, cc * KV + col0: cc * KV + col0 + 128],
                                 wv[:, cc * 512: cc * 512 + 512],
                                 start=(cc == 0), stop=(cc == 3))
            v3 = vt[f][jt][:, :].rearrange("p (h c) -> p h c", c=128)
            p3 = ps[:, :].rearrange("p (h c) -> p h c", c=64)
            nc.vector.tensor_copy(v3[:, :, 0:64], p3[:, :, :])
            nc.vector.memset(v3[:, :, 64:128], 1.0)

        def emit_out_slice(f, tt):
            tg = f * (N // 128) + tt
            ps = pp.tile([128, 512], dt.float32, tag="pp", name="pp_t")
            for hp in range(4):
                nc.tensor.matmul(ps[:],
                                 aoT[hp][:, tg * 128:(tg + 1) * 128],
                                 wo[:, hp * 512:(hp + 1) * 512],
                                 start=(hp == 0), stop=(hp == 3))
            osb = outp.tile([128, 512], dt.float32, tag="osb", name="osb_t")
            nc.scalar.copy(osb[:], ps[:])
            nc.sync.dma_start(out_d[tg * 128:(tg + 1) * 128, :], osb[:])

        def emit_lf_sim(f, iw):
            """Leftover-key sim for ALL (hp, hr) into one [128, 1024] psum
            tile: rows 32*hp + j (j < nlf), cols hr*512 + q. 64x32 array
            tiles so the 8 matmuls run concurrently. One exp drains it."""
            st = simp.tile([128, 1024], dt.float32, tag="sim", name="sim_t")
            # rows the matmuls don't touch must exp() to zero: the av-side
            # contraction runs over all 128 rows against zero-padded vlf
            nc.vector.memset(st[:], -1.0e9)
            q0 = f * N + iw * 512
            for c in range(4):
                for hr in (0, 1):
                    nc.tensor.matmul(
                        st[32 * c: 32 * c + nlf, hr * 512: hr * 512 + 512],
                        klf[64 * hr: 64 * hr + 64,
                            (c * 2 + f) * nlf: (c * 2 + f) * nlf + nlf],
                        qT[c][64 * hr: 64 * hr + 64, q0: q0 + 512],
                        start=True, stop=True,
                        tile_position=(64 * hr, 32 * c))
            et = lfet.tile([128, 1024], dt.bfloat16, tag="lfet", name="lfet_t")
            nc.scalar.activation(et[:, :], st[:, :], AF.Exp)
            if diag and iw in lf_bands:
                wi = lf_bands.index(iw)
                e3 = et[:, :].rearrange("p (h q) -> p h q", h=2)
                m3 = mlf[:, wi * 2 * lf_w:(wi + 1) * 2 * lf_w].rearrange(
                    "p (h w) -> p h w", h=2)
                lo = lf_lo[wi]
                nc.gpsimd.tensor_mul(e3[:, :, lo:lo + lf_w],
                                     e3[:, :, lo:lo + lf_w], m3[:, :, :])
            lf_ET[(f, iw)] = et

        def emit_av_combo(f, hp, ET, hr, iw):
            h = hp * 2 + hr
            ps = avp.tile([128, 512], dt.float32, tag="av", name="av_t")
            for jt in range(njt):
                nc.tensor.matmul(
                    ps[:, :],
                    vt[f][jt][:, 128 * h: 128 * h + 128],
                    ET[jt][:, iw * 1024 + hr * 512: iw * 1024 + hr * 512 + 512],
                    start=(jt == 0), stop=(jt == njt - 1 and not (nlf and LF_ON)))
            if nlf and LF_ON:
                blk = hp * 4 + hr * 2 + f
                nc.tensor.matmul(
                    ps[:, :],
                    vlf[:, blk * 128: blk * 128 + 128],
                    lf_ET[(f, iw)][:, hr * 512: hr * 512 + 512],
                    start=False, stop=True)
            win = slice(f * N + iw * 512, f * N + iw * 512 + 512)
            # den sits replicated at psum rows 64:128; shift-copy it down to
            # lanes 0-63 (verified exact), recip at base 0, one normalize mul
            d0 = smallp.tile([64, 512], dt.float32, tag="d0", name="d0_t")
            nc.vector.tensor_copy(d0[:], ps[64:128, :])
            sr = smallp.tile([64, 512], dt.float32, tag="srec", name="srec_t")
            nc.vector.reciprocal_approx_fast(sr[:], d0[:])
            if hr == 0:
                nc.vector.tensor_mul(aoT[hp][0:64, win], ps[0:64, :], sr[:])
            else:
                sc = smallp.tile([64, 512], dt.bfloat16, tag="aosc", name="aosc_t")
                nc.vector.tensor_mul(sc[:], ps[0:64, :], sr[:])
                nc.sync.dma_start(aoT[hp][64:128, win], sc[:])

        def emit_sim_pair(f, hp, jt, et):
            """One psum tile per (jt, iw) holding BOTH heads side by side
            (cols hr*512+q). The hr0/hr1 matmuls share the tile, so they
            become schedulable at the same instant and issue back-to-back
            into different 64-row PE tiles + different psum banks ->
            concurrent execution. ET columns: iw*1024 + hr*512 + q."""
            k0 = f * nkp + jt * 128
            for iw in (0, 1):
                st = simp.tile([128, 1024], dt.float32, tag="sim", name="sim_t")
                for hr in (0, 1):
                    po = 64 * hr
                    nc.tensor.matmul(
                        st[:, hr * 512: hr * 512 + 512],
                        kT[hp][po:po + 64, k0:k0 + 128],
                        qT[hp][po:po + 64, f * N + iw * 512: f * N + iw * 512 + 512],
                        start=True, stop=True)
                nc.scalar.activation(et[:, iw * 1024:(iw + 1) * 1024],
                                     st[:], AF.Exp)

        def emit_band(jt, et):
            iwj, lo = band_lo[jt]
            e3 = et[:, iwj * 1024:(iwj + 1) * 1024].rearrange(
                "p (h q) -> p h q", h=2)
            m3 = mmb[:, jt * 2 * band_w:(jt + 1) * 2 * band_w].rearrange(
                "p (h w) -> p h w", h=2)
            nc.gpsimd.tensor_mul(e3[:, :, lo:lo + band_w],
                                 e3[:, :, lo:lo + band_w], m3[:, :, :])

        # ---- demand-driven schedule ----
        fills = deque()
        for f in range(FPC):
            for hp in range(4):
                for iw in range(2):
                    fills.append(('q', f, hp, iw))
            for hp in range(4):
                fills.append(('k', f, hp))
            if nlf and LF_ON:
                for iw in range(2):
                    fills.append(('ls', f, iw))
            for jt in range(njt):
                fills.append(('v', f, jt))

        def run_fill(u):
            kind = u[0]
            if kind == 'q':
                emit_q_slice(u[1], u[2], u[3])
            elif kind == 'k':
                emit_k_slice(u[1], u[2])
            elif kind == 'v':
                emit_v_slice(u[1], u[2])
            elif kind == 'ls':
                emit_lf_sim(u[1], u[2])
            elif kind == 'o':
                emit_out_slice(u[1], u[2])

        def drain_matching(pred):
            rest = deque()
            while fills:
                u = fills.popleft()
                if pred(u):
                    run_fill(u)
                else:
                    rest.append(u)
            fills.extend(rest)

        def drain_some(k):
            for _ in range(k):
                if fills:
                    run_fill(fills.popleft())

        prev = None
        groups = [(f, hp) for f in range(FPC) for hp in range(4)]
        for g, (f, hp) in enumerate(groups):
            # prerequisites: q/k slices of this group's (f, hp)
            drain_matching(lambda u: u[0] in ('q', 'k') and u[1] == f
                           and u[2] == hp)
            if hp == 0 and f > 0:
                # frame entry: flush the whole frame's projection fills now
                # so their DVE drain-casts complete before sims/avs need them
                drain_matching(lambda u: u[1] == f)
            if prev is not None:
                # av of prev group needs its frame's v tiles + leftover ET
                # (and 'ls' needs ALL q of that frame, which precede it in
                # the fills queue, so drain 'q' of the frame too)
                drain_matching(lambda u: u[0] in ('q', 'v', 'ls')
                               and u[1] == prev[0])
            ET = {jt: etp.tile([128, 2048], dt.bfloat16, tag="et", name="et_t")
                  for jt in range(njt)}
            for jt in range(njt):
                emit_sim_pair(f, hp, jt, ET[jt])
                if diag:
                    emit_band(jt, ET[jt])
                if prev is not None and jt < 4:
                    pf, php, pET = prev
                    emit_av_combo(pf, php, pET, jt // 2, jt % 2)
                else:
                    drain_some(2)
                drain_some(1)
            if prev is not None and prev[1] == 3:
                for tt in range(N // 128):
                    fills.append(('o', prev[0], tt))
            prev = (f, hp, ET)

        pf, php, pET = prev
        for c in range(4):
            # iw-major: both heads of window iw0 finish first so its
            # out-projections can start while iw1 combos still run
            emit_av_combo(pf, php, pET, c % 2, c // 2)
            drain_some(2)
            if c == 1:
                for tt in range(4):
                    emit_out_slice(pf, tt)
        while fills:
            run_fill(fills.popleft())
        for tt in range(4, N // 128):
            emit_out_slice(pf, tt)

    nc.compile()
    return nc


def _chunk_major(a):
    """[512, M] f32 -> [128, 4*M] bf16, contraction chunk-major."""
    m = a.shape[1]
    return np.ascontiguousarray(
        a.reshape(4, 128, m).transpose(1, 0, 2).reshape(128, 4 * m)).astype(bf16)


lf_lo = None  # set by kernel(); read by _build's band closure


def kernel(x, W_qkv, W_out, mask, diag):
    global lf_lo
    x = np.asarray(x, dtype=np.float32).reshape(F * N, DIM)
    W_qkv = np.asarray(W_qkv, dtype=np.float32)
    W_out = np.asarray(W_out, dtype=np.float32)
    maskv = np.asarray(mask).reshape(N)
    diag = int(np.asarray(diag))

    kept = np.flatnonzero(maskv != 0)
    nk = int(kept.size)
    assert nk > 0, "all-masked input not supported"
    njt = nk // 128
    nlf = nk - njt * 128
    assert njt >= 1 and nlf <= 32, (
        f"kernel specialized for nk with nk%128<=32, got nk={nk}")
    nkp = njt * 128
    kept_main = kept[:nkp]
    lf = kept[nkp:]

    Wq = W_qkv[:, 0:512] * np.float32(D ** -0.5)
    Wk = W_qkv[:, 512:1024]
    Wv = W_qkv[:, 1024:1536]

    if FP8_QK:
        f8 = ml_dtypes.float8_e4m3fn
        S = np.float32(FP8_S)

        def _dr_weights(W_eff):
            # DoubleRow layout: per (g, hp) a [128, 2, 128] block with the
            # contraction chunk PAIR (2g, 2g+1) interleaved along free dim.
            cm = np.ascontiguousarray(W_eff.reshape(4, 128, 512))
            out = np.zeros((128, 4 * 512), np.float32)
            for g in range(2):
                for hp in range(4):
                    base = g * 1024 + hp * 256
                    for j in range(2):
                        out[:, base + j * 128: base + (j + 1) * 128] = \
                            cm[2 * g + j][:, hp * 128:(hp + 1) * 128]
            return (out * S).astype(f8)

        wq_h = _dr_weights(Wq)
        wk_h = _dr_weights(Wk)
    else:
        wq_h = _chunk_major(Wq)
        wk_h = _chunk_major(Wk)
    wv_h = _chunk_major(Wv)
    wo_h = _chunk_major(W_out)

    if diag:
        # each main tile's diag positions must fall inside ONE 512-wide
        # query window (iw); band coords are local to that window
        iws, los, ws = [], [], []
        for jt in range(njt):
            idx = kept_main[jt * 128: jt * 128 + 128]
            iwj = int(idx.min()) // 512
            assert int(idx.max()) // 512 == iwj, "diag band straddles iw window"
            lo = (int(idx.min()) - iwj * 512) & ~1
            iws.append(iwj)
            los.append(lo)
            ws.append(int(idx.max()) - iwj * 512 + 1 - lo)
        bw = (max(ws) + 1) & ~1
        los = [min(lo, 512 - bw) for lo in los]
        mmb_h = np.ones((128, njt * 2 * bw), np.float32)
        for jt in range(njt):
            p = np.arange(128)
            off = kept_main[jt * 128: jt * 128 + 128] - iws[jt] * 512 - los[jt]
            mmb_h[p, jt * 2 * bw + off] = 0.0
            mmb_h[p, jt * 2 * bw + bw + off] = 0.0
        mmb_h = mmb_h.astype(bf16)
        band_lo = tuple(zip(iws, los))
    else:
        bw = 0
        band_lo = None
        mmb_h = None

    # leftover diag-band masks, per iw window that contains leftover keys
    lf_bands = ()
    lf_w = 0
    mlf_h = None
    lf_lo_l = []
    if nlf and diag:
        wins = sorted(set(int(p) // 512 for p in lf))
        lf_bands = tuple(wins)
        spans = []
        for iw in wins:
            loc = [int(p) - iw * 512 for p in lf if int(p) // 512 == iw]
            lo = min(loc) & ~1
            spans.append((lo, max(loc) + 1 - lo))
        lf_w = (max(s[1] for s in spans) + 1) & ~1
        lf_lo_l = [min(s[0], 512 - lf_w) for s in spans]
        mlf_h = np.ones((128, len(wins) * 2 * lf_w), np.float32)
        for wi, iw in enumerate(wins):
            for c in range(4):
                for j, p in enumerate(lf):
                    if int(p) // 512 != iw:
                        continue
                    col = int(p) - iw * 512 - lf_lo_l[wi]
                    mlf_h[32 * c + j, wi * 2 * lf_w + col] = 0.0
                    mlf_h[32 * c + j, wi * 2 * lf_w + lf_w + col] = 0.0
        mlf_h = mlf_h.astype(bf16)

    lf_lo = tuple(lf_lo_l)
    key = (njt, nlf, diag, bw, band_lo, lf_bands, lf_w, lf_lo)
    if key not in _nc_cache:
        _nc_cache[key] = _build(njt, nlf, diag, band_lo, bw, lf_bands, lf_w)
    nc = _nc_cache[key]

    xbf = x.astype(bf16)
    in_maps = []
    for m in range(NCORES):
        xs = xbf[m * T:(m + 1) * T]                      # [T, DIM] bf16
        xsT32 = np.ascontiguousarray(xs.T.astype(np.float32))
        kvrows = np.zeros((FPC * nkp, DIM), np.float32)
        for f in range(FPC):
            kvrows[f * nkp: f * nkp + nkp] = xs[f * N + kept_main].astype(np.float32)
        kvT32 = np.ascontiguousarray(kvrows.T)
        if FP8_QK:
            f8 = ml_dtypes.float8_e4m3fn
            xm32 = x[m * T:(m + 1) * T]                  # fp32 source
            xT_h = np.ascontiguousarray(xm32.T).reshape(4, 128, T).transpose(
                1, 0, 2).reshape(128, 4 * T).astype(f8)
            kv32 = np.zeros((FPC * nkp, DIM), np.float32)
            for f in range(FPC):
                kv32[f * nkp: f * nkp + nkp] = xm32[f * N + kept_main]
            xkv8_h = np.ascontiguousarray(kv32.T).reshape(4, 128, FPC * nkp
                ).transpose(1, 0, 2).reshape(128, 4 * FPC * nkp).astype(f8)
        else:
            xT_h = _chunk_major(xsT32)
        xkvT_h = _chunk_major(kvT32)
        im = dict(xT=xT_h, xkvT=xkvT_h, wq=wq_h, wk=wk_h, wv=wv_h, wo=wo_h)
        if FP8_QK:
            im["xkv8"] = xkv8_h
        if diag:
            im["mmb"] = mmb_h
        if nlf:
            xm = x[m * T:(m + 1) * T]                    # fp32 source
            klf_h = np.zeros((128, 8 * nlf), np.float32)
            vlf_h = np.zeros((128, 16 * 128), np.float32)
            for f in range(FPC):
                xl = xm[f * N + lf]                      # [nlf, 512]
                kp = xl @ Wk                             # [nlf, 512]
                vp = xl @ Wv
                for c in range(4):
                    for hr in range(2):
                        h = 2 * c + hr
                        blk = (c * 2 + f) * nlf
                        klf_h[64 * hr:64 * hr + 64, blk:blk + nlf] = \
                            kp[:, h * 64:(h + 1) * 64].T
                        b = c * 4 + hr * 2 + f
                        vlf_h[32 * c:32 * c + nlf, b * 128:b * 128 + 64] = \
                            vp[:, h * 64:(h + 1) * 64]
                        vlf_h[32 * c:32 * c + nlf, b * 128 + 64:b * 128 + 128] = 1.0
            im["klf"] = klf_h.astype(bf16)
            im["vlf"] = vlf_h.astype(bf16)
            if diag and lf_bands:
                im["mlf"] = mlf_h
        in_maps.append(im)

    core_ids = list(range(NCORES))
    if TRACE:
        r = run_bass_kernel_spmd(nc, in_maps, core_ids, trace=True,
                                 tmpdir=TRACE_TMPDIR)
        LAST["exec_time_ns"] = r.exec_time_ns
        LAST["results"] = r
        results = r.results
    else:
        results = None
        for attempt in range(3):
            try:
                results = run_bass_kernel_spmd(nc, in_maps, core_ids).results
                break
            except Exception:
                if attempt == 2:
                    raise
                import time as _time
                _time.sleep(2.0)

    out = np.concatenate([np.asarray(results[m]["out"]) for m in range(NCORES)],
                         axis=0)
    return out.reshape(B, F * N, DIM).astype(np.float32)


# revision 54
# speedup vs baseline: 1.0231x; 1.0231x over previous
"""Trainium2 Bass kernel for nn_Attention_22179211117150 (sparse axial attention).

Strategy (8 NeuronCores, zero collectives); HW ~142us vs 173us baseline:
  - Axial attention: tokens attend within their own frame (N=1024 tokens,
    F=16 frames). 2 frames per core; weights replicated; fully local.
  - Keys/values compressed on host to the kept (mask!=0) positions.
  - nk=516 kept keys split as 4 FULL 128-key tiles + nlf=4 leftover keys.
    Leftover k/v projections are computed on host (tiny); leftover sim for
    all (hp, hr) packs into ONE [128, 1024] psum tile via 64x32 PE array
    tiling (rows 32*hp+j, cols hr*512+q), so ONE exp covers what used to
    take 16 activations. Leftover psum is memset to -1e9 first so unused
    rows exp to 0, letting the leftover av ride a standard K=128 matmul
    against zero-padded vlf (no mid-group array-mode switch).
  - All matmuls bf16 (fp32 psum); softmax exp in f32 on ScalarE.
  - Transposed dataflow: qT/kT [d, tokens], simT [keys, queries].
  - Sim pairs: one [128, 1024] psum tile per (jt, iw) holds BOTH heads
    side by side (cols hr*512+q); the hr0/hr1 matmuls share the tile so
    they become ready together and issue back-to-back into different
    64-row PE tiles + banks -> concurrent (Dstart ~4ns, ~2x sim).
  - Softmax denominators: the av lhsT carries the v dims in cols 0:64 and
    ALL-ONES in cols 64:128, so den lands REPLICATED at psum rows 64:128
    for free (matmuls are N-bound; M=128 costs the same as M=65). A
    partition-shift tensor_copy (rows 64:128 -> lanes 0:63, verified
    exact) + reciprocal_approx_fast + one normalize mul finish it — no
    broadcast matmul, no DMA bounce.
    (Pitfalls found: reciprocal_approx_fast misreads PSUM rows by one
    element and breaks at nonzero SBUF base partitions — keep its input
    at SBUF partition 0.)
  - Diagonal mask: narrow band multiply on GpSimd over a [128, 2, bw]
    strided view of ET covering both heads in one instruction; band
    coords are local to the 512-wide iw window of each key tile.
  - Demand-driven emission: projection/out-proj/av work is queued as fill
    units and drained between sim key-tiles so the in-order PE queue always
    has work while ScalarE chews exps. fp8 DoubleRow q/k projections were
    measured SLOWER (160us, DR LDWEIGHTS penalty + mode switches) and are
    off; steady-state PE runs at the 216ns/MM N=512 roofline.
"""
import numpy as np
import ml_dtypes
from collections import deque
from contextlib import ExitStack

import concourse.bass as bass
import concourse.mybir as mybir
import concourse.tile as tile
from concourse import bacc
from concourse.bass_utils import run_bass_kernel_spmd

dt = mybir.dt
AF = mybir.ActivationFunctionType
bf16 = ml_dtypes.bfloat16

B, F, N, H, D, DIM = 1, 16, 1024, 8, 64, 512
NCORES = 8
FPC = F // NCORES          # frames per core
T = FPC * N                # tokens per core

TRACE = False
TRACE_TMPDIR = None
LAST = {}

import os as _os
DEN_DMA = _os.environ.get("KDEN_DMA", "1") == "1"
LF_ON = _os.environ.get("KLF_ON", "1") == "1"
FP8_QK = _os.environ.get("KFP8", "0") == "1"   # fp8 DoubleRow q/k: passes
# (1.62e-2) but measured SLOWER (160us vs 145us): DR LDWEIGHTS penalty +
# mode switches outweigh the halved MM count. Keep off.
FP8_S = 64.0               # weight pre-scale so fp8 values stay normal

_nc_cache = {}


def _build(njt, nlf, diag, band_lo, band_w, lf_bands, lf_w):
    """njt full 128-key tiles + nlf (<=32) leftover keys per frame.

    lf_bands: tuple of iw windows (0/1) that contain leftover diag
    positions; lf_w: band width of the leftover diag mask."""
    nkp = njt * 128
    KV = FPC * nkp                     # kv rows per core (no padding)
    nc = bacc.Bacc("TRN2", target_bir_lowering=False, debug=False,
                   num_devices=NCORES)

    qk_dt = dt.float8e4 if FP8_QK else dt.bfloat16
    xT_d = nc.declare_dram_parameter("xT", [128, 4 * T], qk_dt, isOutput=False)
    wq_d = nc.declare_dram_parameter("wq", [128, 4 * 512], qk_dt, isOutput=False)
    wk_d = nc.declare_dram_parameter("wk", [128, 4 * 512], qk_dt, isOutput=False)
    if FP8_QK:
        xkv8_d = nc.declare_dram_parameter("xkv8", [128, 4 * KV], dt.float8e4,
                                           isOutput=False)
    xkvT_d = nc.declare_dram_parameter("xkvT", [128, 4 * KV], dt.bfloat16, isOutput=False)
    wv_d = nc.declare_dram_parameter("wv", [128, 4 * 512], dt.bfloat16, isOutput=False)
    wo_d = nc.declare_dram_parameter("wo", [128, 4 * 512], dt.bfloat16, isOutput=False)
    if diag:
        mmb_d = nc.declare_dram_parameter("mmb", [128, njt * 2 * band_w],
                                          dt.bfloat16, isOutput=False)
    if nlf:
        klf_d = nc.declare_dram_parameter("klf", [128, 8 * nlf], dt.bfloat16,
                                          isOutput=False)
        # vlf: one [128, 128] column block per (hp, hr, f) — cols 0:64 the
        # v dims, cols 64:128 all-ones (den replication); only rows
        # 32*hp .. 32*hp+nlf are nonzero, so a standard K=128 matmul picks
        # out just that hp's leftover keys (zero rows annihilate the rest).
        vlf_d = nc.declare_dram_parameter("vlf", [128, 16 * 128], dt.bfloat16,
                                          isOutput=False)
        if diag and lf_bands:
            mlf_d = nc.declare_dram_parameter(
                "mlf", [128, len(lf_bands) * 2 * lf_w], dt.bfloat16,
                isOutput=False)
    out_d = nc.declare_dram_parameter("out", [T, DIM], dt.float32, isOutput=True)

    with tile.TileContext(nc) as tc, ExitStack() as ctx:
        consts = ctx.enter_context(tc.tile_pool(name="consts", bufs=1))
        work = ctx.enter_context(tc.tile_pool(name="work", bufs=1))
        etp = ctx.enter_context(tc.tile_pool(name="etp", bufs=12))
        lfet = ctx.enter_context(tc.tile_pool(name="lfet", bufs=4))
        smallp = ctx.enter_context(tc.tile_pool(name="small", bufs=6))
        outp = ctx.enter_context(tc.tile_pool(name="outp", bufs=3))
        dramp = ctx.enter_context(tc.tile_pool(name="dramp", bufs=4, space="DRAM"))
        simp = ctx.enter_context(tc.tile_pool(name="simp", bufs=2, space="PSUM"))
        avp = ctx.enter_context(tc.tile_pool(name="avp", bufs=2, space="PSUM"))
        pp = ctx.enter_context(tc.tile_pool(name="pp", bufs=2, space="PSUM"))

        def load(d, shape, dtype, tag, eng=None, frame_split=False):
            eng = eng or nc.sync
            t = consts.tile(shape, dtype, tag=tag, name=tag)
            n = shape[1]
            if frame_split:
                # 4 contraction chunks x FPC frame-halves; frame-0 halves
                # first so frame-0 projections start as early as possible.
                half = n // 8
                for fh in range(2):
                    for cc in range(4):
                        o = cc * (n // 4) + fh * half
                        eng.dma_start(t[:, o:o + half], d[:, o:o + half])
            else:
                eng.dma_start(t[:], d[:])
            return t

        # queue programs ordered so the FIRST group's inputs stream first:
        #   sync:   wq, xT-f0, xT-f1, wo
        #   scalar: wk, xkvT-f0, wv, xkvT-f1
        #   gpsimd: mmb, klf, vlf
        wq = load(wq_d, [128, 4 * 512], qk_dt, "wq")
        wk = load(wk_d, [128, 4 * 512], qk_dt, "wk", eng=nc.scalar)
        xT = load(xT_d, [128, 4 * T], qk_dt, "xT", frame_split=True)
        if FP8_QK:
            xkv8 = load(xkv8_d, [128, 4 * KV], dt.float8e4, "xkv8",
                        eng=nc.gpsimd, frame_split=True)
        xkvT = consts.tile([128, 4 * KV], dt.bfloat16, tag="xkvT", name="xkvT")
        kvh = KV // 2
        for cc in range(4):
            nc.scalar.dma_start(xkvT[:, cc * KV: cc * KV + kvh],
                                xkvT_d[:, cc * KV: cc * KV + kvh])
        wv = load(wv_d, [128, 4 * 512], dt.bfloat16, "wv", eng=nc.scalar)
        for cc in range(4):
            nc.scalar.dma_start(xkvT[:, cc * KV + kvh: (cc + 1) * KV],
                                xkvT_d[:, cc * KV + kvh: (cc + 1) * KV])
        wo = load(wo_d, [128, 4 * 512], dt.bfloat16, "wo")
        if diag:
            mmb = load(mmb_d, [128, njt * 2 * band_w], dt.bfloat16, "mmb",
                       eng=nc.gpsimd)
        if nlf:
            klf = load(klf_d, [128, 8 * nlf], dt.bfloat16, "klf", eng=nc.gpsimd)
            vlf = load(vlf_d, [128, 16 * 128], dt.bfloat16, "vlf", eng=nc.gpsimd)
            if diag and lf_bands:
                mlf = load(mlf_d, [128, len(lf_bands) * 2 * lf_w], dt.bfloat16,
                           "mlf", eng=nc.gpsimd)

        ones_sb = work.tile([128, 64], dt.bfloat16, tag="ones", name="ones")
        nc.vector.memset(ones_sb[:], 1.0)

        # PE warm-up burst while inputs stream in (HAM clock gate).
        warm_src = work.tile([128, 512], dt.bfloat16, tag="warmsrc", name="warmsrc")
        nc.vector.memset(warm_src[:], 0.5)
        wps = pp.tile([128, 512], dt.float32, tag="pp", name="pp_t")
        for wi in range(12):
            nc.tensor.matmul(wps[0:64, :], ones_sb[:, 0:64], warm_src[:],
                             start=(wi == 0), stop=(wi == 11))
        wsb = smallp.tile([1, 64], dt.float32, tag="warm", name="warm_t")
        nc.vector.tensor_copy(wsb[:], wps[0:1, 0:64])
        wdr = dramp.tile([1, 64], dt.float32, tag="wdr", name="wdr_t")
        nc.sync.dma_start(wdr[:], wsb[:])

        qT = [work.tile([128, T], dt.bfloat16, tag=f"qT{hp}", name=f"qT{hp}")
              for hp in range(4)]
        kT = [work.tile([128, KV], dt.bfloat16, tag=f"kT{hp}", name=f"kT{hp}")
              for hp in range(4)]
        # per head h: cols 128h..128h+64 = v dims, cols 128h+64..128h+128 =
        # ones. The av matmul then yields av at psum rows 0:64 AND the
        # softmax denominator REPLICATED at rows 64:128 — no broadcast step.
        vt = [[work.tile([128, 8 * 128], dt.bfloat16, tag=f"v{f}_{jt}",
                         name=f"v{f}_{jt}") for jt in range(njt)]
              for f in range(FPC)]
        aoT = [work.tile([128, T], dt.bfloat16, tag=f"aoT{hp}", name=f"aoT{hp}")
               for hp in range(4)]

        lf_ET = {}                     # (f, iw) -> [128, 1024] bf16 tile

        # ---- emitters ----

        DR = mybir.MatmulPerfMode.DoubleRow

        def emit_q_slice(f, hp, iw):
            w0 = f * N + iw * 512
            ps = pp.tile([128, 512], dt.float32, tag="pp", name="pp_t")
            if FP8_QK:
                for g in (0, 1):
                    w3 = wq[:, g * 1024 + hp * 256: g * 1024 + hp * 256 + 256
                            ].rearrange("p (j m) -> p j m", j=2)
                    x3 = xT[:, g * 2 * T: (g + 1) * 2 * T
                            ].rearrange("p (j t) -> p j t", j=2)
                    nc.tensor.matmul(ps[:], w3[:, :, :], x3[:, :, w0:w0 + 512],
                                     start=(g == 0), stop=(g == 1),
                                     perf_mode=DR)
                nc.vector.tensor_scalar_mul(qT[hp][:, w0:w0 + 512], ps[:],
                                            1.0 / FP8_S)
            else:
                for cc in range(4):
                    nc.tensor.matmul(
                        ps[:],
                        wq[:, cc * 512 + hp * 128: cc * 512 + hp * 128 + 128],
                        xT[:, cc * T + w0: cc * T + w0 + 512],
                        start=(cc == 0), stop=(cc == 3))
                nc.vector.tensor_copy(qT[hp][:, w0:w0 + 512], ps[:])

        def emit_k_slice(f, hp):
            c0 = f * nkp
            ps = pp.tile([128, 512], dt.float32, tag="pp", name="pp_t")
            if FP8_QK:
                for g in (0, 1):
                    w3 = wk[:, g * 1024 + hp * 256: g * 1024 + hp * 256 + 256
                            ].rearrange("p (j m) -> p j m", j=2)
                    x3 = xkv8[:, g * 2 * KV: (g + 1) * 2 * KV
                              ].rearrange("p (j t) -> p j t", j=2)
                    nc.tensor.matmul(ps[:, 0:nkp], w3[:, :, :],
                                     x3[:, :, c0:c0 + nkp],
                                     start=(g == 0), stop=(g == 1),
                                     perf_mode=DR)
                nc.vector.tensor_scalar_mul(kT[hp][:, c0:c0 + nkp],
                                            ps[:, 0:nkp], 1.0 / FP8_S)
            else:
                for cc in range(4):
                    nc.tensor.matmul(
                        ps[:, 0:nkp],
                        wk[:, cc * 512 + hp * 128: cc * 512 + hp * 128 + 128],
                        xkvT[:, cc * KV + c0: cc * KV + c0 + nkp],
                        start=(cc == 0), stop=(cc == 3))
                nc.vector.tensor_copy(kT[hp][:, c0:c0 + nkp], ps[:, 0:nkp])

        def emit_v_slice(f, jt):
            col0 = f * nkp + jt * 128
            ps = pp.tile([128, 512], dt.float32, tag="pp", name="pp_t")
            for cc in range(4):
                nc.tensor.matmul(ps[:],
                                 xkvT[:, cc * KV + col0: cc * KV + col0 + 128],
                                 wv[:, cc * 512: cc * 512 + 512],
                                 start=(cc == 0), stop=(cc == 3))
            v3 = vt[f][jt][:, :].rearrange("p (h c) -> p h c", c=128)
            p3 = ps[:, :].rearrange("p (h c) -> p h c", c=64)
            nc.vector.tensor_copy(v3[:, :, 0:64], p3[:, :, :])
            nc.vector.memset(v3[:, :, 64:128], 1.0)

        def emit_out_slice(f, tt):
            tg = f * (N // 128) + tt
            ps = pp.tile([128, 512], dt.float32, tag="pp", name="pp_t")
            for hp in range(4):
                nc.tensor.matmul(ps[:],
                                 aoT[hp][:, tg * 128:(tg + 1) * 128],
                                 wo[:, hp * 512:(hp + 1) * 512],
                                 start=(hp == 0), stop=(hp == 3))
            osb = outp.tile([128, 512], dt.float32, tag="osb", name="osb_t")
            # alternate drain engine so out-copies don't queue between the
            # exps on ScalarE (which would delay the next key-tile's sims)
            if tt % 2 == 0:
                nc.scalar.copy(osb[:], ps[:])
            else:
                nc.vector.tensor_copy(osb[:], ps[:])
            nc.sync.dma_start(out_d[tg * 128:(tg + 1) * 128, :], osb[:])

        def emit_lf_sim(f, iw):
            """Leftover-key sim for ALL (hp, hr) into one [128, 1024] psum
            tile: rows 32*hp + j (j < nlf), cols hr*512 + q. 64x32 array
            tiles so the 8 matmuls run concurrently. One exp drains it."""
            st = simp.tile([128, 1024], dt.float32, tag="sim", name="sim_t")
            # rows the matmuls don't touch must exp() to zero: the av-side
            # contraction runs over all 128 rows against zero-padded vlf
            nc.vector.memset(st[:], -1.0e9)
            q0 = f * N + iw * 512
            for c in range(4):
                for hr in (0, 1):
                    nc.tensor.matmul(
                        st[32 * c: 32 * c + nlf, hr * 512: hr * 512 + 512],
                        klf[64 * hr: 64 * hr + 64,
                            (c * 2 + f) * nlf: (c * 2 + f) * nlf + nlf],
                        qT[c][64 * hr: 64 * hr + 64, q0: q0 + 512],
                        start=True, stop=True,
                        tile_position=(64 * hr, 32 * c))
            et = lfet.tile([128, 1024], dt.bfloat16, tag="lfet", name="lfet_t")
            nc.scalar.activation(et[:, :], st[:, :], AF.Exp)
            if diag and iw in lf_bands:
                wi = lf_bands.index(iw)
                e3 = et[:, :].rearrange("p (h q) -> p h q", h=2)
                m3 = mlf[:, wi * 2 * lf_w:(wi + 1) * 2 * lf_w].rearrange(
                    "p (h w) -> p h w", h=2)
                lo = lf_lo[wi]
                nc.gpsimd.tensor_mul(e3[:, :, lo:lo + lf_w],
                                     e3[:, :, lo:lo + lf_w], m3[:, :, :])
            lf_ET[(f, iw)] = et

        def emit_av_combo(f, hp, ET, hr, iw):
            h = hp * 2 + hr
            ps = avp.tile([128, 512], dt.float32, tag="av", name="av_t")
            for jt in range(njt):
                nc.tensor.matmul(
                    ps[:, :],
                    vt[f][jt][:, 128 * h: 128 * h + 128],
                    ET[jt][:, iw * 1024 + hr * 512: iw * 1024 + hr * 512 + 512],
                    start=(jt == 0), stop=(jt == njt - 1 and not (nlf and LF_ON)))
            if nlf and LF_ON:
                blk = hp * 4 + hr * 2 + f
                nc.tensor.matmul(
                    ps[:, :],
                    vlf[:, blk * 128: blk * 128 + 128],
                    lf_ET[(f, iw)][:, hr * 512: hr * 512 + 512],
                    start=False, stop=True)
            win = slice(f * N + iw * 512, f * N + iw * 512 + 512)
            # den sits replicated at psum rows 64:128; shift-copy it down to
            # lanes 0-63 (verified exact), recip at base 0, one normalize mul
            d0 = smallp.tile([64, 512], dt.float32, tag="d0", name="d0_t")
            nc.vector.tensor_copy(d0[:], ps[64:128, :])
            sr = smallp.tile([64, 512], dt.float32, tag="srec", name="srec_t")
            nc.vector.reciprocal_approx_fast(sr[:], d0[:])
            if hr == 0:
                nc.vector.tensor_mul(aoT[hp][0:64, win], ps[0:64, :], sr[:])
            else:
                sc = smallp.tile([64, 512], dt.bfloat16, tag="aosc", name="aosc_t")
                nc.vector.tensor_mul(sc[:], ps[0:64, :], sr[:])
                nc.sync.dma_start(aoT[hp][64:128, win], sc[:])

        def emit_sim_pair(f, hp, jt, et):
            """One psum tile per (jt, iw) holding BOTH heads side by side
            (cols hr*512+q). The hr0/hr1 matmuls share the tile, so they
            become schedulable at the same instant and issue back-to-back
            into different 64-row PE tiles + different psum banks ->
            concurrent execution. ET columns: iw*1024 + hr*512 + q."""
            k0 = f * nkp + jt * 128
            for iw in (0, 1):
                st = simp.tile([128, 1024], dt.float32, tag="sim", name="sim_t")
                for hr in (0, 1):
                    po = 64 * hr
                    nc.tensor.matmul(
                        st[:, hr * 512: hr * 512 + 512],
                        kT[hp][po:po + 64, k0:k0 + 128],
                        qT[hp][po:po + 64, f * N + iw * 512: f * N + iw * 512 + 512],
                        start=True, stop=True)
                nc.scalar.activation(et[:, iw * 1024:(iw + 1) * 1024],
                                     st[:], AF.Exp)

        def emit_band(jt, et):
            iwj, lo = band_lo[jt]
            e3 = et[:, iwj * 1024:(iwj + 1) * 1024].rearrange(
                "p (h q) -> p h q", h=2)
            m3 = mmb[:, jt * 2 * band_w:(jt + 1) * 2 * band_w].rearrange(
                "p (h w) -> p h w", h=2)
            nc.gpsimd.tensor_mul(e3[:, :, lo:lo + band_w],
                                 e3[:, :, lo:lo + band_w], m3[:, :, :])

        # ---- demand-driven schedule ----
        fills = deque()
        for f in range(FPC):
            for hp in range(4):
                for iw in range(2):
                    fills.append(('q', f, hp, iw))
            for hp in range(4):
                fills.append(('k', f, hp))
            if nlf and LF_ON:
                for iw in range(2):
                    fills.append(('ls', f, iw))
            for jt in range(njt):
                fills.append(('v', f, jt))

        def run_fill(u):
            kind = u[0]
            if kind == 'q':
                emit_q_slice(u[1], u[2], u[3])
            elif kind == 'k':
                emit_k_slice(u[1], u[2])
            elif kind == 'v':
                emit_v_slice(u[1], u[2])
            elif kind == 'ls':
                emit_lf_sim(u[1], u[2])
            elif kind == 'o':
                emit_out_slice(u[1], u[2])

        def drain_matching(pred):
            rest = deque()
            while fills:
                u = fills.popleft()
                if pred(u):
                    run_fill(u)
                else:
                    rest.append(u)
            fills.extend(rest)

        def drain_some(k):
            for _ in range(k):
                if fills:
                    run_fill(fills.popleft())

        prev = None
        late_o = deque()
        groups = [(f, hp) for f in range(FPC) for hp in range(4)]
        for g, (f, hp) in enumerate(groups):
            if late_o and hp >= 2:
                # release reserved out-proj work into the tail groups (2 per
                # group), which otherwise run dry of fills and go ACT-paced
                for _ in range(2):
                    if late_o:
                        fills.append(late_o.popleft())
            # prerequisites: q/k slices of this group's (f, hp)
            drain_matching(lambda u: u[0] in ('q', 'k') and u[1] == f
                           and u[2] == hp)
            if hp == 0 and f > 0:
                # frame entry: flush the whole frame's projection fills now
                # so their DVE drain-casts complete before sims/avs need them
                drain_matching(lambda u: u[1] == f)
            if prev is not None:
                # av of prev group needs its frame's v tiles + leftover ET
                # (and 'ls' needs ALL q of that frame, which precede it in
                # the fills queue, so drain 'q' of the frame too)
                drain_matching(lambda u: u[0] in ('q', 'v', 'ls')
                               and u[1] == prev[0])
            ET = {jt: etp.tile([128, 2048], dt.bfloat16, tag="et", name="et_t")
                  for jt in range(njt)}
            for jt in range(njt):
                emit_sim_pair(f, hp, jt, ET[jt])
                if diag:
                    emit_band(jt, ET[jt])
                if prev is not None and jt < 4:
                    pf, php, pET = prev
                    emit_av_combo(pf, php, pET, jt // 2, jt % 2)
                else:
                    drain_some(2)
                drain_some(1)
            if prev is not None and prev[1] == 3:
                for tt in range(4):
                    fills.append(('o', prev[0], tt))
                for tt in range(4, N // 128):
                    late_o.append(('o', prev[0], tt))
            prev = (f, hp, ET)

        pf, php, pET = prev
        for c in range(4):
            # iw-major: both heads of window iw0 finish first so its
            # out-projections can start while iw1 combos still run
            emit_av_combo(pf, php, pET, c % 2, c // 2)
            drain_some(2)
            if c == 1:
                for tt in range(4):
                    emit_out_slice(pf, tt)
        while fills:
            run_fill(fills.popleft())
        for tt in range(4, N // 128):
            emit_out_slice(pf, tt)

    nc.compile()
    return nc


def _chunk_major(a):
    """[512, M] f32 -> [128, 4*M] bf16, contraction chunk-major."""
    m = a.shape[1]
    return np.ascontiguousarray(
        a.reshape(4, 128, m).transpose(1, 0, 2).reshape(128, 4 * m)).astype(bf16)


lf_lo = None  # set by kernel(); read by _build's band closure


def kernel(x, W_qkv, W_out, mask, diag):
    global lf_lo
    x = np.asarray(x, dtype=np.float32).reshape(F * N, DIM)
    W_qkv = np.asarray(W_qkv, dtype=np.float32)
    W_out = np.asarray(W_out, dtype=np.float32)
    maskv = np.asarray(mask).reshape(N)
    diag = int(np.asarray(diag))

    kept = np.flatnonzero(maskv != 0)
    nk = int(kept.size)
    assert nk > 0, "all-masked input not supported"
    njt = nk // 128
    nlf = nk - njt * 128
    assert njt >= 1 and nlf <= 32, (
        f"kernel specialized for nk with nk%128<=32, got nk={nk}")
    nkp = njt * 128
    kept_main = kept[:nkp]
    lf = kept[nkp:]

    Wq = W_qkv[:, 0:512] * np.float32(D ** -0.5)
    Wk = W_qkv[:, 512:1024]
    Wv = W_qkv[:, 1024:1536]

    if FP8_QK:
        f8 = ml_dtypes.float8_e4m3fn
        S = np.float32(FP8_S)

        def _dr_weights(W_eff):
            # DoubleRow layout: per (g, hp) a [128, 2, 128] block with the
            # contraction chunk PAIR (2g, 2g+1) interleaved along free dim.
            cm = np.ascontiguousarray(W_eff.reshape(4, 128, 512))
            out = np.zeros((128, 4 * 512), np.float32)
            for g in range(2):
                for hp in range(4):
                    base = g * 1024 + hp * 256
                    for j in range(2):
                        out[:, base + j * 128: base + (j + 1) * 128] = \
                            cm[2 * g + j][:, hp * 128:(hp + 1) * 128]
            return (out * S).astype(f8)

        wq_h = _dr_weights(Wq)
        wk_h = _dr_weights(Wk)
    else:
        wq_h = _chunk_major(Wq)
        wk_h = _chunk_major(Wk)
    wv_h = _chunk_major(Wv)
    wo_h = _chunk_major(W_out)

    if diag:
        # each main tile's diag positions must fall inside ONE 512-wide
        # query window (iw); band coords are local to that window
        iws, los, ws = [], [], []
        for jt in range(njt):
            idx = kept_main[jt * 128: jt * 128 + 128]
            iwj = int(idx.min()) // 512
            assert int(idx.max()) // 512 == iwj, "diag band straddles iw window"
            lo = (int(idx.min()) - iwj * 512) & ~1
            iws.append(iwj)
            los.append(lo)
            ws.append(int(idx.max()) - iwj * 512 + 1 - lo)
        bw = (max(ws) + 1) & ~1
        los = [min(lo, 512 - bw) for lo in los]
        mmb_h = np.ones((128, njt * 2 * bw), np.float32)
        for jt in range(njt):
            p = np.arange(128)
            off = kept_main[jt * 128: jt * 128 + 128] - iws[jt] * 512 - los[jt]
            mmb_h[p, jt * 2 * bw + off] = 0.0
            mmb_h[p, jt * 2 * bw + bw + off] = 0.0
        mmb_h = mmb_h.astype(bf16)
        band_lo = tuple(zip(iws, los))
    else:
        bw = 0
        band_lo = None
        mmb_h = None

    # leftover diag-band masks, per iw window that contains leftover keys
    lf_bands = ()
    lf_w = 0
    mlf_h = None
    lf_lo_l = []
    if nlf and diag:
        wins = sorted(set(int(p) // 512 for p in lf))
        lf_bands = tuple(wins)
        spans = []
        for iw in wins:
            loc = [int(p) - iw * 512 for p in lf if int(p) // 512 == iw]
            lo = min(loc) & ~1
            spans.append((lo, max(loc) + 1 - lo))
        lf_w = (max(s[1] for s in spans) + 1) & ~1
        lf_lo_l = [min(s[0], 512 - lf_w) for s in spans]
        mlf_h = np.ones((128, len(wins) * 2 * lf_w), np.float32)
        for wi, iw in enumerate(wins):
            for c in range(4):
                for j, p in enumerate(lf):
                    if int(p) // 512 != iw:
                        continue
                    col = int(p) - iw * 512 - lf_lo_l[wi]
                    mlf_h[32 * c + j, wi * 2 * lf_w + col] = 0.0
                    mlf_h[32 * c + j, wi * 2 * lf_w + lf_w + col] = 0.0
        mlf_h = mlf_h.astype(bf16)

    lf_lo = tuple(lf_lo_l)
    key = (njt, nlf, diag, bw, band_lo, lf_bands, lf_w, lf_lo)
    if key not in _nc_cache:
        _nc_cache[key] = _build(njt, nlf, diag, band_lo, bw, lf_bands, lf_w)
    nc = _nc_cache[key]

    xbf = x.astype(bf16)
    in_maps = []
    for m in range(NCORES):
        xs = xbf[m * T:(m + 1) * T]                      # [T, DIM] bf16
        xsT32 = np.ascontiguousarray(xs.T.astype(np.float32))
        kvrows = np.zeros((FPC * nkp, DIM), np.float32)
        for f in range(FPC):
            kvrows[f * nkp: f * nkp + nkp] = xs[f * N + kept_main].astype(np.float32)
        kvT32 = np.ascontiguousarray(kvrows.T)
        if FP8_QK:
            f8 = ml_dtypes.float8_e4m3fn
            xm32 = x[m * T:(m + 1) * T]                  # fp32 source
            xT_h = np.ascontiguousarray(xm32.T).reshape(4, 128, T).transpose(
                1, 0, 2).reshape(128, 4 * T).astype(f8)
            kv32 = np.zeros((FPC * nkp, DIM), np.float32)
            for f in range(FPC):
                kv32[f * nkp: f * nkp + nkp] = xm32[f * N + kept_main]
            xkv8_h = np.ascontiguousarray(kv32.T).reshape(4, 128, FPC * nkp
                ).transpose(1, 0, 2).reshape(128, 4 * FPC * nkp).astype(f8)
        else:
            xT_h = _chunk_major(xsT32)
        xkvT_h = _chunk_major(kvT32)
        im = dict(xT=xT_h, xkvT=xkvT_h, wq=wq_h, wk=wk_h, wv=wv_h, wo=wo_h)
        if FP8_QK:
            im["xkv8"] = xkv8_h
        if diag:
            im["mmb"] = mmb_h
        if nlf:
            xm = x[m * T:(m + 1) * T]                    # fp32 source
            klf_h = np.zeros((128, 8 * nlf), np.float32)
            vlf_h = np.zeros((128, 16 * 128), np.float32)
            for f in range(FPC):
                xl = xm[f * N + lf]                      # [nlf, 512]
                kp = xl @ Wk                             # [nlf, 512]
                vp = xl @ Wv
                for c in range(4):
                    for hr in range(2):
                        h = 2 * c + hr
                        blk = (c * 2 + f) * nlf
                        klf_h[64 * hr:64 * hr + 64, blk:blk + nlf] = \
                            kp[:, h * 64:(h + 1) * 64].T
                        b = c * 4 + hr * 2 + f
                        vlf_h[32 * c:32 * c + nlf, b * 128:b * 128 + 64] = \
                            vp[:, h * 64:(h + 1) * 64]
                        vlf_h[32 * c:32 * c + nlf, b * 128 + 64:b * 128 + 128] = 1.0
            im["klf"] = klf_h.astype(bf16)
            im["vlf"] = vlf_h.astype(bf16)
            if diag and lf_bands:
                im["mlf"] = mlf_h
        in_maps.append(im)

    core_ids = list(range(NCORES))
    if TRACE:
        r = run_bass_kernel_spmd(nc, in_maps, core_ids, trace=True,
                                 tmpdir=TRACE_TMPDIR)
        LAST["exec_time_ns"] = r.exec_time_ns
        LAST["results"] = r
        results = r.results
    else:
        results = None
        for attempt in range(3):
            try:
                results = run_bass_kernel_spmd(nc, in_maps, core_ids).results
                break
            except Exception:
                if attempt == 2:
                    raise
                import time as _time
                _time.sleep(2.0)

    out = np.concatenate([np.asarray(results[m]["out"]) for m in range(NCORES)],
                         axis=0)
    return out.reshape(B, F * N, DIM).astype(np.float32)
